# revision 5
# baseline (speedup 1.0000x reference)
"""Trainium2 Bass kernel for nn_CrossAttention_9174050144362.

Reference computation (per batch b, spatial flattened to hw=4096):
    Q = Wq @ a + bq      [128, 4096]
    K = Wk @ p + bk      [128, 4096]
    V = Wv @ p + bv      [256, 4096]
    attn = softmax_n(Q^T K)            [4096, 4096]
    out  = V @ attn^T + a              [256, 4096]

Sharding: 8 cores = (4 batches) x (2 query halves of 2048). Each core
computes full K/V for its batch (recomputed per half; ~6% extra flops)
and attends its 2048 queries against all 4096 keys. No collectives.

Per-core kernel strategy (compute-bound, PE-centric):
  * All matmuls in float32r (FP22 truncated fp32) -> 1 cycle/row on PE.
  * S is computed TRANSPOSED: S^T[n, m] tiles via matmul(lhsT=K_tile,
    rhs=Q_chunk), so the softmax reduction over n is a partition-dim
    reduction. P^T = exp(S^T) goes straight from PSUM through ACT to
    SBUF, and feeds matmul(lhsT=V^T_tile, rhs=P^T) accumulation - no
    transposes of the big 4096x2048 matrix anywhere.
  * The softmax denominator is obtained by accumulating P^T tiles on
    the (otherwise idle) Vector engine, then one matmul with an
    all-ones [128,128] lhsT which simultaneously reduces over the
    partition dim and broadcasts the row-sums to all 128 partitions.
  * bv is folded into the epilogue: sum_n attn = 1, so V-bias adds
    bv[c] to every output pixel (a must stay pristine for the Q proj).
"""

import numpy as np

import concourse.bass as bass
import concourse.tile as tile
from concourse import bacc, mybir
from concourse.bass_utils import run_bass_kernel_spmd

B, C, H, W = 4, 256, 64, 64
HW = H * W            # 4096 keys
CH = C // 2           # 128 q/k channels
P = 128               # partitions
MS = HW // 2          # 2048 queries per core
MCH = 512             # query chunk (PSUM-bank sized)
NT = HW // P          # 32 key tiles
NCORES = 8

F32 = mybir.dt.float32
F32R = mybir.dt.float32r
AF = mybir.ActivationFunctionType

# Module-level knobs for the dev harness (test.py); harmless defaults for
# the grading path which just calls kernel(**inputs).
TRACE = False
TMPDIR = None
LAST_RESULT = None

_PROG = None
_ONES = np.ones((P, P), dtype=np.float32)


def _emit(tc, out_d, a_d, p_d, wqt_d, wkt_d, wvt_d, bq_d, bk_d, bv_d, ones_d):
    nc = tc.nc
    ts = bass.ts

    with (
        tc.tile_pool(name="statics", bufs=1) as statics,
        tc.tile_pool(name="ptp", bufs=4) as ptp,
        tc.tile_pool(name="accp", bufs=2) as accp,
        tc.tile_pool(name="rcp", bufs=2) as rcp,
        tc.tile_pool(name="osb", bufs=3) as osb,
        tc.tile_pool(name="psA", bufs=3, space="PSUM") as psA,
        tc.tile_pool(name="psOut", bufs=1, space="PSUM") as psOut,
        tc.tile_pool(name="psDen", bufs=2, space="PSUM") as psDen,
    ):
        # ---- statics into SBUF (c split as c = co*128 + ci, ci on partitions)
        a_sb = statics.tile([P, 2, MS], F32R)
        nc.sync.dma_start(a_sb[:], a_d.rearrange("(co ci) m -> ci co m", ci=P))
        p_sb = statics.tile([P, 2, HW], F32R)
        nc.sync.dma_start(p_sb[:], p_d.rearrange("(co ci) m -> ci co m", ci=P))
        wqt_sb = statics.tile([P, 2, CH], F32R)
        nc.sync.dma_start(wqt_sb[:], wqt_d.rearrange("(co ci) o -> ci co o", ci=P))
        wkt_sb = statics.tile([P, 2, CH], F32R)
        nc.sync.dma_start(wkt_sb[:], wkt_d.rearrange("(co ci) o -> ci co o", ci=P))
        wvt_sb = statics.tile([P, 2, C], F32R)
        nc.sync.dma_start(wvt_sb[:], wvt_d.rearrange("(co ci) o -> ci co o", ci=P))
        bq_sb = statics.tile([P, 1], F32)
        nc.sync.dma_start(bq_sb[:], bq_d[:])
        bk_sb = statics.tile([P, 1], F32)
        nc.sync.dma_start(bk_sb[:], bk_d[:])
        bv_sb = statics.tile([P, 2], F32)
        nc.sync.dma_start(bv_sb[:], bv_d[:])
        ones_sb = statics.tile([P, P], F32R)
        nc.sync.dma_start(ones_sb[:], ones_d[:])

        # ---- projections
        q_sb = statics.tile([P, MS], F32R)
        k_sb = statics.tile([P, HW], F32R)
        vt_sb = statics.tile([P, NT, C], F32R)

        wqt_r = wqt_sb
        wkt_r = wkt_sb
        wvt_r = wvt_sb
        a_r = a_sb
        p_r = p_sb

        for t in range(MS // MCH):
            ps_q = psA.tile([P, MCH], F32, tag="ps")
            for co in range(2):
                nc.tensor.matmul(
                    ps_q[:], wqt_r[:, co, :], a_r[:, co, ts(t, MCH)],
                    start=(co == 0), stop=(co == 1),
                )
            nc.scalar.activation(
                q_sb[:, ts(t, MCH)], ps_q[:], AF.Identity, bias=bq_sb[:, 0:1]
            )
        for t in range(HW // MCH):
            ps_k = psA.tile([P, MCH], F32, tag="ps")
            for co in range(2):
                nc.tensor.matmul(
                    ps_k[:], wkt_r[:, co, :], p_r[:, co, ts(t, MCH)],
                    start=(co == 0), stop=(co == 1),
                )
            nc.scalar.activation(
                k_sb[:, ts(t, MCH)], ps_k[:], AF.Identity, bias=bk_sb[:, 0:1]
            )
        # V^T tiles: vt[n, c] = sum_ci p[ci, n] * WvT[ci, c]   (no bias)
        for t in range(NT):
            ps_v = psA.tile([P, C], F32, tag="ps")
            for co in range(2):
                nc.tensor.matmul(
                    ps_v[:], p_r[:, co, ts(t, P)], wvt_r[:, co, :],
                    start=(co == 0), stop=(co == 1),
                )
            nc.scalar.copy(vt_sb[:, t, :], ps_v[:])

        q_r = q_sb
        k_r = k_sb
        vt_r = vt_sb
        ones_r = ones_sb

        out_v = out_d.rearrange("(co ci) m -> ci co m", ci=P)

        # ---- attention main loop over query chunks
        for mc in range(MS // MCH):
            out_ps = psOut.tile([P, 2, MCH], F32)
            acc = accp.tile([P, MCH], F32R)
            for t in range(NT):
                s_ps = psA.tile([P, MCH], F32, tag="ps")
                nc.tensor.matmul(
                    s_ps[:], k_r[:, ts(t, P)], q_r[:, ts(mc, MCH)],
                    start=True, stop=True,
                )
                pt = ptp.tile([P, MCH], F32R)
                nc.scalar.activation(pt[:], s_ps[:], AF.Exp)
                if t == 0:
                    nc.vector.tensor_copy(acc[:], pt[:])
                else:
                    nc.vector.tensor_add(acc[:], acc[:], pt[:])
                pt_r = pt
                for co in range(2):
                    nc.tensor.matmul(
                        out_ps[:, co, :], vt_r[:, t, ts(co, P)], pt_r[:],
                        start=(t == 0), stop=(t == NT - 1),
                    )
            # denominator: ones^T @ acc reduces over n AND broadcasts to
            # all partitions in a single matmul
            den_ps = psDen.tile([P, MCH], F32)
            nc.tensor.matmul(
                den_ps[:], ones_r[:], acc[:], start=True, stop=True
            )
            recip = rcp.tile([P, MCH], F32)
            nc.vector.reciprocal(recip[:], den_ps[:])
            for co in range(2):
                o_sb = osb.tile([P, MCH], F32, tag="osb")
                nc.vector.tensor_mul(o_sb[:], out_ps[:, co, :], recip[:])
                nc.vector.tensor_add(o_sb[:], o_sb[:], a_sb[:, co, ts(mc, MCH)])
                # bv folded here: attn rows sum to 1, so V-bias is +bv[c]
                nc.vector.tensor_scalar_add(o_sb[:], o_sb[:], bv_sb[:, co : co + 1])
                nc.sync.dma_start(out_v[:, co, ts(mc, MCH)], o_sb[:])


def _build():
    nc = bacc.Bacc("TRN2", target_bir_lowering=False, debug=False)
    a_d = nc.dram_tensor("a_s", [C, MS], F32R, kind="ExternalInput").ap()
    p_d = nc.dram_tensor("p_s", [C, HW], F32R, kind="ExternalInput").ap()
    wqt_d = nc.dram_tensor("wqt", [C, CH], F32R, kind="ExternalInput").ap()
    wkt_d = nc.dram_tensor("wkt", [C, CH], F32R, kind="ExternalInput").ap()
    wvt_d = nc.dram_tensor("wvt", [C, C], F32R, kind="ExternalInput").ap()
    bq_d = nc.dram_tensor("bq", [CH, 1], F32, kind="ExternalInput").ap()
    bk_d = nc.dram_tensor("bk", [CH, 1], F32, kind="ExternalInput").ap()
    bv_d = nc.dram_tensor("bv", [P, 2], F32, kind="ExternalInput").ap()
    ones_d = nc.dram_tensor("onesm", [P, P], F32R, kind="ExternalInput").ap()
    out_d = nc.dram_tensor("out_s", [C, MS], F32, kind="ExternalOutput").ap()
    with tile.TileContext(nc) as tc:
        _emit(tc, out_d, a_d, p_d, wqt_d, wkt_d, wvt_d, bq_d, bk_d, bv_d, ones_d)
    nc.compile()
    return nc


def _get_prog():
    global _PROG
    if _PROG is None:
        _PROG = _build()
    return _PROG


def kernel(**inputs):
    a = np.ascontiguousarray(np.asarray(inputs["a"], dtype=np.float32)).reshape(
        B, C, HW
    )
    p = np.ascontiguousarray(np.asarray(inputs["p"], dtype=np.float32)).reshape(
        B, C, HW
    )
    wqt = np.ascontiguousarray(np.asarray(inputs["Wq"], dtype=np.float32).T)
    wkt = np.ascontiguousarray(np.asarray(inputs["Wk"], dtype=np.float32).T)
    wvt = np.ascontiguousarray(np.asarray(inputs["Wv"], dtype=np.float32).T)
    bq = np.ascontiguousarray(np.asarray(inputs["bq"], dtype=np.float32)).reshape(
        CH, 1
    )
    bk = np.ascontiguousarray(np.asarray(inputs["bk"], dtype=np.float32)).reshape(
        CH, 1
    )
    bv = np.ascontiguousarray(
        np.asarray(inputs["bv"], dtype=np.float32).reshape(2, P).T
    )

    nc = _get_prog()
    in_maps = []
    for core in range(NCORES):
        b, h = divmod(core, 2)
        in_maps.append(
            {
                "a_s": np.ascontiguousarray(a[b, :, h * MS : (h + 1) * MS]),
                "p_s": p[b],
                "wqt": wqt,
                "wkt": wkt,
                "wvt": wvt,
                "bq": bq,
                "bk": bk,
                "bv": bv,
                "onesm": _ONES,
            }
        )
    kwargs = {}
    if TRACE:
        kwargs["trace"] = True
        if TMPDIR:
            kwargs["tmpdir"] = TMPDIR
    res = run_bass_kernel_spmd(nc, in_maps, core_ids=list(range(NCORES)), **kwargs)
    global LAST_RESULT
    LAST_RESULT = res

    out = np.empty((B, C, HW), dtype=np.float32)
    for core in range(NCORES):
        b, h = divmod(core, 2)
        out[b, :, h * MS : (h + 1) * MS] = res.results[core]["out_s"]
    return out.reshape(B, C, H, W)


# revision 6
# speedup vs baseline: 1.0181x; 1.0181x over previous
"""Trainium2 Bass kernel for nn_CrossAttention_9174050144362.

Reference computation (per batch b, spatial flattened to hw=4096):
    Q = Wq @ a + bq      [128, 4096]
    K = Wk @ p + bk      [128, 4096]
    V = Wv @ p + bv      [256, 4096]
    attn = softmax_n(Q^T K)            [4096, 4096]
    out  = V @ attn^T + a              [256, 4096]

Sharding: 8 cores = (4 batches) x (2 query halves of 2048). Each core
computes full K/V for its batch (recomputed per half; ~6% extra flops)
and attends its 2048 queries against all 4096 keys. No collectives.

Per-core kernel strategy (compute-bound, PE-centric):
  * All matmuls in float32r (FP22 truncated fp32) -> 1 cycle/row on PE.
  * S is computed TRANSPOSED: S^T[n, m] tiles via matmul(lhsT=K_tile,
    rhs=Q_chunk), so the softmax reduction over n is a partition-dim
    reduction. P^T = exp(S^T) goes straight from PSUM through ACT to
    SBUF, and feeds matmul(lhsT=V^T_tile, rhs=P^T) accumulation - no
    transposes of the big 4096x2048 matrix anywhere.
  * The softmax denominator is obtained by accumulating P^T tiles on
    the (otherwise idle) Vector engine, then one matmul with an
    all-ones [128,128] lhsT which simultaneously reduces over the
    partition dim and broadcasts the row-sums to all 128 partitions.
  * bv is folded into the epilogue: sum_n attn = 1, so V-bias adds
    bv[c] to every output pixel (a must stay pristine for the Q proj).
"""

import numpy as np

import concourse.bass as bass
import concourse.tile as tile
from concourse import bacc, mybir
from concourse.bass_utils import run_bass_kernel_spmd

B, C, H, W = 4, 256, 64, 64
HW = H * W            # 4096 keys
CH = C // 2           # 128 q/k channels
P = 128               # partitions
MS = HW // 2          # 2048 queries per core
MCH = 512             # query chunk (PSUM-bank sized)
NT = HW // P          # 32 key tiles
NCORES = 8

F32 = mybir.dt.float32
F32R = mybir.dt.float32r
AF = mybir.ActivationFunctionType

# Module-level knobs for the dev harness (test.py); harmless defaults for
# the grading path which just calls kernel(**inputs).
TRACE = False
TMPDIR = None
LAST_RESULT = None

_PROG = None
_ONES = np.ones((P, P), dtype=np.float32)


def _emit(tc, out_d, a_d, p_d, wqt_d, wkt_d, wvt_d, bq_d, bk_d, bv_d, ones_d):
    nc = tc.nc
    ts = bass.ts

    with (
        tc.tile_pool(name="statics", bufs=1) as statics,
        tc.tile_pool(name="ptp", bufs=6) as ptp,
        tc.tile_pool(name="accp", bufs=2) as accp,
        tc.tile_pool(name="rcp", bufs=2) as rcp,
        tc.tile_pool(name="osb", bufs=3) as osb,
        tc.tile_pool(name="psA", bufs=3, space="PSUM") as psA,
        tc.tile_pool(name="psOut", bufs=2, space="PSUM") as psOut,
        tc.tile_pool(name="psDen", bufs=1, space="PSUM") as psDen,
    ):
        # ---- statics into SBUF (c split as c = co*128 + ci, ci on partitions)
        # a/p are DMA'd in m-chunks so projections can start before the full
        # tensor lands (Tile tracks subtile deps).
        a_v = a_d.rearrange("(co ci) m -> ci co m", ci=P)
        a_sb = statics.tile([P, 2, MS], F32R)
        for h in range(2):
            nc.sync.dma_start(a_sb[:, :, ts(h, MS // 2)], a_v[:, :, ts(h, MS // 2)])
        p_v = p_d.rearrange("(co ci) m -> ci co m", ci=P)
        p_sb = statics.tile([P, 2, HW], F32R)
        for h in range(4):
            nc.sync.dma_start(p_sb[:, :, ts(h, HW // 4)], p_v[:, :, ts(h, HW // 4)])
        wqt_sb = statics.tile([P, 2, CH], F32R)
        nc.sync.dma_start(wqt_sb[:], wqt_d.rearrange("(co ci) o -> ci co o", ci=P))
        wkt_sb = statics.tile([P, 2, CH], F32R)
        nc.sync.dma_start(wkt_sb[:], wkt_d.rearrange("(co ci) o -> ci co o", ci=P))
        wvt_sb = statics.tile([P, 2, C], F32R)
        nc.sync.dma_start(wvt_sb[:], wvt_d.rearrange("(co ci) o -> ci co o", ci=P))
        bq_sb = statics.tile([P, 1], F32)
        nc.sync.dma_start(bq_sb[:], bq_d[:])
        bk_sb = statics.tile([P, 1], F32)
        nc.sync.dma_start(bk_sb[:], bk_d[:])
        bv_sb = statics.tile([P, 2], F32)
        nc.sync.dma_start(bv_sb[:], bv_d[:])
        ones_sb = statics.tile([P, P], F32R)
        nc.sync.dma_start(ones_sb[:], ones_d[:])

        # ---- projections
        q_sb = statics.tile([P, MS], F32R)
        k_sb = statics.tile([P, HW], F32R)
        vt_sb = statics.tile([P, NT, C], F32R)

        for t in range(MS // MCH):
            ps_q = psA.tile([P, MCH], F32, tag="ps")
            for co in range(2):
                nc.tensor.matmul(
                    ps_q[:], wqt_sb[:, co, :], a_sb[:, co, ts(t, MCH)],
                    start=(co == 0), stop=(co == 1),
                )
            nc.scalar.activation(
                q_sb[:, ts(t, MCH)], ps_q[:], AF.Identity, bias=bq_sb[:, 0:1]
            )
        for t in range(HW // MCH):
            ps_k = psA.tile([P, MCH], F32, tag="ps")
            for co in range(2):
                nc.tensor.matmul(
                    ps_k[:], wkt_sb[:, co, :], p_sb[:, co, ts(t, MCH)],
                    start=(co == 0), stop=(co == 1),
                )
            nc.scalar.activation(
                k_sb[:, ts(t, MCH)], ps_k[:], AF.Identity, bias=bk_sb[:, 0:1]
            )
        # V^T tiles: vt[n, c] = sum_ci p[ci, n] * WvT[ci, c]   (no bias)
        for t in range(NT):
            ps_v = psA.tile([P, C], F32, tag="ps")
            for co in range(2):
                nc.tensor.matmul(
                    ps_v[:], p_sb[:, co, ts(t, P)], wvt_sb[:, co, :],
                    start=(co == 0), stop=(co == 1),
                )
            nc.scalar.copy(vt_sb[:, t, :], ps_v[:])

        out_v = out_d.rearrange("(co ci) m -> ci co m", ci=P)

        def epilogue(mc, out_ps, acc):
            # denominator: ones^T @ acc reduces over n AND broadcasts to
            # all partitions in a single matmul
            den_ps = psDen.tile([P, MCH], F32, tag="den")
            nc.tensor.matmul(den_ps[:], ones_sb[:], acc[:], start=True, stop=True)
            recip = rcp.tile([P, MCH], F32, tag="rc")
            nc.vector.reciprocal(recip[:], den_ps[:])
            for co in range(2):
                o_sb = osb.tile([P, MCH], F32, tag="osb")
                nc.vector.tensor_mul(o_sb[:], out_ps[:, co, :], recip[:])
                nc.vector.tensor_add(o_sb[:], o_sb[:], a_sb[:, co, ts(mc, MCH)])
                # bv folded here: attn rows sum to 1, so V-bias is +bv[c]
                nc.vector.tensor_scalar_add(o_sb[:], o_sb[:], bv_sb[:, co : co + 1])
                nc.sync.dma_start(out_v[:, co, ts(mc, MCH)], o_sb[:])

        # ---- attention main loop over query chunks; the epilogue of chunk
        # k is emitted in the middle of chunk k+1 so the PE stream never
        # stalls on the DVE denominator-accumulation chain.
        pending = None
        for mc in range(MS // MCH):
            out_ps = psOut.tile([P, 2, MCH], F32, tag="out")
            acc = accp.tile([P, MCH], F32R, tag="acc")
            prev_pt = None
            for t in range(NT):
                s_ps = psA.tile([P, MCH], F32, tag="ps")
                nc.tensor.matmul(
                    s_ps[:], k_sb[:, ts(t, P)], q_sb[:, ts(mc, MCH)],
                    start=True, stop=True,
                )
                pt = ptp.tile([P, MCH], F32R, tag="pt")
                nc.scalar.activation(pt[:], s_ps[:], AF.Exp)
                if t == 0:
                    nc.vector.tensor_copy(acc[:], pt[:])
                else:
                    nc.vector.tensor_add(acc[:], acc[:], pt[:])
                # V.P matmuls run one iteration behind the S matmul so the
                # PE never waits on the exp of the tile it just produced.
                if prev_pt is not None:
                    tp, ptp_prev = prev_pt
                    for co in range(2):
                        nc.tensor.matmul(
                            out_ps[:, co, :], vt_sb[:, tp, ts(co, P)], ptp_prev[:],
                            start=(tp == 0), stop=False,
                        )
                prev_pt = (t, pt)
                if t == 4 and pending is not None:
                    epilogue(*pending)
                    pending = None
            tp, ptp_prev = prev_pt
            for co in range(2):
                nc.tensor.matmul(
                    out_ps[:, co, :], vt_sb[:, tp, ts(co, P)], ptp_prev[:],
                    start=False, stop=True,
                )
            pending = (mc, out_ps, acc)
        epilogue(*pending)


def _build():
    nc = bacc.Bacc("TRN2", target_bir_lowering=False, debug=False)
    a_d = nc.dram_tensor("a_s", [C, MS], F32R, kind="ExternalInput").ap()
    p_d = nc.dram_tensor("p_s", [C, HW], F32R, kind="ExternalInput").ap()
    wqt_d = nc.dram_tensor("wqt", [C, CH], F32R, kind="ExternalInput").ap()
    wkt_d = nc.dram_tensor("wkt", [C, CH], F32R, kind="ExternalInput").ap()
    wvt_d = nc.dram_tensor("wvt", [C, C], F32R, kind="ExternalInput").ap()
    bq_d = nc.dram_tensor("bq", [CH, 1], F32, kind="ExternalInput").ap()
    bk_d = nc.dram_tensor("bk", [CH, 1], F32, kind="ExternalInput").ap()
    bv_d = nc.dram_tensor("bv", [P, 2], F32, kind="ExternalInput").ap()
    ones_d = nc.dram_tensor("onesm", [P, P], F32R, kind="ExternalInput").ap()
    out_d = nc.dram_tensor("out_s", [C, MS], F32, kind="ExternalOutput").ap()
    with tile.TileContext(nc) as tc:
        _emit(tc, out_d, a_d, p_d, wqt_d, wkt_d, wvt_d, bq_d, bk_d, bv_d, ones_d)
    nc.compile()
    return nc


def _get_prog():
    global _PROG
    if _PROG is None:
        _PROG = _build()
    return _PROG


def kernel(**inputs):
    a = np.ascontiguousarray(np.asarray(inputs["a"], dtype=np.float32)).reshape(
        B, C, HW
    )
    p = np.ascontiguousarray(np.asarray(inputs["p"], dtype=np.float32)).reshape(
        B, C, HW
    )
    wqt = np.ascontiguousarray(np.asarray(inputs["Wq"], dtype=np.float32).T)
    wkt = np.ascontiguousarray(np.asarray(inputs["Wk"], dtype=np.float32).T)
    wvt = np.ascontiguousarray(np.asarray(inputs["Wv"], dtype=np.float32).T)
    bq = np.ascontiguousarray(np.asarray(inputs["bq"], dtype=np.float32)).reshape(
        CH, 1
    )
    bk = np.ascontiguousarray(np.asarray(inputs["bk"], dtype=np.float32)).reshape(
        CH, 1
    )
    bv = np.ascontiguousarray(
        np.asarray(inputs["bv"], dtype=np.float32).reshape(2, P).T
    )

    nc = _get_prog()
    in_maps = []
    for core in range(NCORES):
        b, h = divmod(core, 2)
        in_maps.append(
            {
                "a_s": np.ascontiguousarray(a[b, :, h * MS : (h + 1) * MS]),
                "p_s": p[b],
                "wqt": wqt,
                "wkt": wkt,
                "wvt": wvt,
                "bq": bq,
                "bk": bk,
                "bv": bv,
                "onesm": _ONES,
            }
        )
    kwargs = {}
    if TRACE:
        kwargs["trace"] = True
        if TMPDIR:
            kwargs["tmpdir"] = TMPDIR
    res = run_bass_kernel_spmd(nc, in_maps, core_ids=list(range(NCORES)), **kwargs)
    global LAST_RESULT
    LAST_RESULT = res

    out = np.empty((B, C, HW), dtype=np.float32)
    for core in range(NCORES):
        b, h = divmod(core, 2)
        out[b, :, h * MS : (h + 1) * MS] = res.results[core]["out_s"]
    return out.reshape(B, C, H, W)


# revision 7
# speedup vs baseline: 1.0507x; 1.0320x over previous
"""Trainium2 Bass kernel for nn_CrossAttention_9174050144362.

Reference computation (per batch b, spatial flattened to hw=4096):
    Q = Wq @ a + bq      [128, 4096]
    K = Wk @ p + bk      [128, 4096]
    V = Wv @ p + bv      [256, 4096]
    attn = softmax_n(Q^T K)            [4096, 4096]
    out  = V @ attn^T + a              [256, 4096]

Sharding: 8 cores = (4 batches) x (2 query halves of 2048). Each core
computes full K/V for its batch (recomputed per half; ~6% extra flops)
and attends its 2048 queries against all 4096 keys. No collectives.

Per-core kernel strategy (compute-bound, PE-centric):
  * All matmuls in float32r (FP22 truncated fp32) -> 1 cycle/row on PE.
  * S is computed TRANSPOSED: S^T[n, m] tiles via matmul(lhsT=K_tile,
    rhs=Q_chunk), so the softmax reduction over n is a partition-dim
    reduction. P^T = exp(S^T) goes straight from PSUM through ACT to
    SBUF, and feeds matmul(lhsT=V^T_tile, rhs=P^T) accumulation - no
    transposes of the big 4096x2048 matrix anywhere.
  * The softmax denominator is obtained by accumulating P^T tiles on
    the (otherwise idle) Vector engine, then one matmul with an
    all-ones [128,128] lhsT which simultaneously reduces over the
    partition dim and broadcasts the row-sums to all 128 partitions.
  * bv is folded into the epilogue: sum_n attn = 1, so V-bias adds
    bv[c] to every output pixel (a must stay pristine for the Q proj).
"""

import numpy as np

import concourse.bass as bass
import concourse.tile as tile
from concourse import bacc, mybir
from concourse.bass_utils import run_bass_kernel_spmd

B, C, H, W = 4, 256, 64, 64
HW = H * W            # 4096 keys
CH = C // 2           # 128 q/k channels
P = 128               # partitions
MS = HW // 2          # 2048 queries per core
MCH = 512             # query chunk (PSUM-bank sized)
NT = HW // P          # 32 key tiles
NCORES = 8

F32 = mybir.dt.float32
F32R = mybir.dt.float32r
AF = mybir.ActivationFunctionType

# Module-level knobs for the dev harness (test.py); harmless defaults for
# the grading path which just calls kernel(**inputs).
TRACE = False
TMPDIR = None
LAST_RESULT = None

_PROG = None
_ONES = np.ones((P, P), dtype=np.float32)


def _emit(tc, out_d, a_d, p_d, wqt_d, wkt_d, wvt_d, bq_d, bk_d, bv_d, ones_d):
    nc = tc.nc
    ts = bass.ts

    with (
        tc.tile_pool(name="statics", bufs=1) as statics,
        tc.tile_pool(name="ptp", bufs=6) as ptp,
        tc.tile_pool(name="accp", bufs=2) as accp,
        tc.tile_pool(name="rcp", bufs=2) as rcp,
        tc.tile_pool(name="osb", bufs=3) as osb,
        tc.tile_pool(name="psA", bufs=3, space="PSUM") as psA,
        tc.tile_pool(name="psOut", bufs=2, space="PSUM") as psOut,
        tc.tile_pool(name="psDen", bufs=1, space="PSUM") as psDen,
    ):
        # ---- statics into SBUF (c split as c = co*128 + ci, ci on partitions)
        # a/p are DMA'd in m-chunks so projections can start before the full
        # tensor lands (Tile tracks subtile deps).
        # small weight/bias DMAs FIRST so projections unblock within ~1us;
        # the bulk a/p loads stream behind them in m-chunks (subtile deps).
        wqt_sb = statics.tile([P, 2, CH], F32R)
        nc.sync.dma_start(wqt_sb[:], wqt_d.rearrange("(co ci) o -> ci co o", ci=P))
        wkt_sb = statics.tile([P, 2, CH], F32R)
        nc.sync.dma_start(wkt_sb[:], wkt_d.rearrange("(co ci) o -> ci co o", ci=P))
        wvt_sb = statics.tile([P, 2, C], F32R)
        nc.sync.dma_start(wvt_sb[:], wvt_d.rearrange("(co ci) o -> ci co o", ci=P))
        bq_sb = statics.tile([P, 1], F32)
        nc.sync.dma_start(bq_sb[:], bq_d[:])
        bk_sb = statics.tile([P, 1], F32)
        nc.sync.dma_start(bk_sb[:], bk_d[:])
        bv_sb = statics.tile([P, 2], F32)
        nc.sync.dma_start(bv_sb[:], bv_d[:])
        ones_sb = statics.tile([P, P], F32R)
        nc.sync.dma_start(ones_sb[:], ones_d[:])
        a_v = a_d.rearrange("(co ci) m -> ci co m", ci=P)
        a_sb = statics.tile([P, 2, MS], F32R)
        for h in range(4):
            nc.sync.dma_start(a_sb[:, :, ts(h, MS // 4)], a_v[:, :, ts(h, MS // 4)])
        p_v = p_d.rearrange("(co ci) m -> ci co m", ci=P)
        p_sb = statics.tile([P, 2, HW], F32R)
        for h in range(8):
            nc.sync.dma_start(p_sb[:, :, ts(h, HW // 8)], p_v[:, :, ts(h, HW // 8)])

        # ---- projections
        q_sb = statics.tile([P, MS], F32R)
        k_sb = statics.tile([P, HW], F32R)
        vt_sb = statics.tile([P, NT, C], F32R)

        for t in range(MS // MCH):
            ps_q = psA.tile([P, MCH], F32, tag="ps")
            for co in range(2):
                nc.tensor.matmul(
                    ps_q[:], wqt_sb[:, co, :], a_sb[:, co, ts(t, MCH)],
                    start=(co == 0), stop=(co == 1),
                )
            nc.scalar.activation(
                q_sb[:, ts(t, MCH)], ps_q[:], AF.Identity, bias=bq_sb[:, 0:1]
            )
        for t in range(HW // MCH):
            ps_k = psA.tile([P, MCH], F32, tag="ps")
            for co in range(2):
                nc.tensor.matmul(
                    ps_k[:], wkt_sb[:, co, :], p_sb[:, co, ts(t, MCH)],
                    start=(co == 0), stop=(co == 1),
                )
            nc.scalar.activation(
                k_sb[:, ts(t, MCH)], ps_k[:], AF.Identity, bias=bk_sb[:, 0:1]
            )
        # V^T tiles: vt[n, c] = sum_ci p[ci, n] * WvT[ci, c]   (no bias)
        for t in range(NT):
            ps_v = psA.tile([P, C], F32, tag="ps")
            for co in range(2):
                nc.tensor.matmul(
                    ps_v[:], p_sb[:, co, ts(t, P)], wvt_sb[:, co, :],
                    start=(co == 0), stop=(co == 1),
                )
            nc.scalar.copy(vt_sb[:, t, :], ps_v[:])

        out_v = out_d.rearrange("(co ci) m -> ci co m", ci=P)

        def epilogue_den(acc):
            # denominator: ones^T @ acc reduces over n AND broadcasts to
            # all partitions in a single matmul
            den_ps = psDen.tile([P, MCH], F32, tag="den")
            nc.tensor.matmul(den_ps[:], ones_sb[:], acc[:], start=True, stop=True)
            recip = rcp.tile([P, MCH], F32, tag="rc")
            nc.vector.reciprocal(recip[:], den_ps[:])
            return recip

        def epilogue_out(mc, out_ps, recip):
            for co in range(2):
                o_sb = osb.tile([P, MCH], F32, tag="osb")
                nc.vector.tensor_mul(o_sb[:], out_ps[:, co, :], recip[:])
                nc.vector.tensor_add(o_sb[:], o_sb[:], a_sb[:, co, ts(mc, MCH)])
                # bv folded here: attn rows sum to 1, so V-bias is +bv[c]
                nc.vector.tensor_scalar_add(o_sb[:], o_sb[:], bv_sb[:, co : co + 1])
                nc.sync.dma_start(out_v[:, co, ts(mc, MCH)], o_sb[:])

        # ---- attention main loop over query chunks. Chunk k's
        # denominator matmul+reciprocal run early in chunk k+1 (t==2), its
        # normalize/store runs at t==6, so the PE stream never stalls on
        # the DVE accumulation chain; the final chunk pays a short tail.
        pending_den = None
        pending_out = None
        for mc in range(MS // MCH):
            out_ps = psOut.tile([P, 2, MCH], F32, tag="out")
            acc = accp.tile([P, MCH], F32R, tag="acc")
            prev_pt = None
            for t in range(NT):
                s_ps = psA.tile([P, MCH], F32, tag="ps")
                nc.tensor.matmul(
                    s_ps[:], k_sb[:, ts(t, P)], q_sb[:, ts(mc, MCH)],
                    start=True, stop=True,
                )
                pt = ptp.tile([P, MCH], F32R, tag="pt")
                nc.scalar.activation(pt[:], s_ps[:], AF.Exp)
                if t == 0:
                    nc.vector.tensor_copy(acc[:], pt[:])
                else:
                    nc.vector.tensor_add(acc[:], acc[:], pt[:])
                # V.P matmuls run one iteration behind the S matmul so the
                # PE never waits on the exp of the tile it just produced.
                if prev_pt is not None:
                    tp, pt_prev = prev_pt
                    for co in range(2):
                        nc.tensor.matmul(
                            out_ps[:, co, :], vt_sb[:, tp, ts(co, P)], pt_prev[:],
                            start=(tp == 0), stop=False,
                        )
                prev_pt = (t, pt)
                if t == 2 and pending_den is not None:
                    pmc, pout, pacc = pending_den
                    pending_out = (pmc, pout, epilogue_den(pacc))
                    pending_den = None
                if t == 6 and pending_out is not None:
                    epilogue_out(*pending_out)
                    pending_out = None
            tp, pt_prev = prev_pt
            for co in range(2):
                nc.tensor.matmul(
                    out_ps[:, co, :], vt_sb[:, tp, ts(co, P)], pt_prev[:],
                    start=False, stop=True,
                )
            pending_den = (mc, out_ps, acc)
        pmc, pout, pacc = pending_den
        epilogue_out(pmc, pout, epilogue_den(pacc))


def _build():
    nc = bacc.Bacc("TRN2", target_bir_lowering=False, debug=False)
    a_d = nc.dram_tensor("a_s", [C, MS], F32R, kind="ExternalInput").ap()
    p_d = nc.dram_tensor("p_s", [C, HW], F32R, kind="ExternalInput").ap()
    wqt_d = nc.dram_tensor("wqt", [C, CH], F32R, kind="ExternalInput").ap()
    wkt_d = nc.dram_tensor("wkt", [C, CH], F32R, kind="ExternalInput").ap()
    wvt_d = nc.dram_tensor("wvt", [C, C], F32R, kind="ExternalInput").ap()
    bq_d = nc.dram_tensor("bq", [CH, 1], F32, kind="ExternalInput").ap()
    bk_d = nc.dram_tensor("bk", [CH, 1], F32, kind="ExternalInput").ap()
    bv_d = nc.dram_tensor("bv", [P, 2], F32, kind="ExternalInput").ap()
    ones_d = nc.dram_tensor("onesm", [P, P], F32R, kind="ExternalInput").ap()
    out_d = nc.dram_tensor("out_s", [C, MS], F32, kind="ExternalOutput").ap()
    with tile.TileContext(nc) as tc:
        _emit(tc, out_d, a_d, p_d, wqt_d, wkt_d, wvt_d, bq_d, bk_d, bv_d, ones_d)
    nc.compile()
    return nc


def _get_prog():
    global _PROG
    if _PROG is None:
        _PROG = _build()
    return _PROG


def kernel(**inputs):
    a = np.ascontiguousarray(np.asarray(inputs["a"], dtype=np.float32)).reshape(
        B, C, HW
    )
    p = np.ascontiguousarray(np.asarray(inputs["p"], dtype=np.float32)).reshape(
        B, C, HW
    )
    wqt = np.ascontiguousarray(np.asarray(inputs["Wq"], dtype=np.float32).T)
    wkt = np.ascontiguousarray(np.asarray(inputs["Wk"], dtype=np.float32).T)
    wvt = np.ascontiguousarray(np.asarray(inputs["Wv"], dtype=np.float32).T)
    bq = np.ascontiguousarray(np.asarray(inputs["bq"], dtype=np.float32)).reshape(
        CH, 1
    )
    bk = np.ascontiguousarray(np.asarray(inputs["bk"], dtype=np.float32)).reshape(
        CH, 1
    )
    bv = np.ascontiguousarray(
        np.asarray(inputs["bv"], dtype=np.float32).reshape(2, P).T
    )

    nc = _get_prog()
    in_maps = []
    for core in range(NCORES):
        b, h = divmod(core, 2)
        in_maps.append(
            {
                "a_s": np.ascontiguousarray(a[b, :, h * MS : (h + 1) * MS]),
                "p_s": p[b],
                "wqt": wqt,
                "wkt": wkt,
                "wvt": wvt,
                "bq": bq,
                "bk": bk,
                "bv": bv,
                "onesm": _ONES,
            }
        )
    kwargs = {}
    if TRACE:
        kwargs["trace"] = True
        if TMPDIR:
            kwargs["tmpdir"] = TMPDIR
    res = run_bass_kernel_spmd(nc, in_maps, core_ids=list(range(NCORES)), **kwargs)
    global LAST_RESULT
    LAST_RESULT = res

    out = np.empty((B, C, HW), dtype=np.float32)
    for core in range(NCORES):
        b, h = divmod(core, 2)
        out[b, :, h * MS : (h + 1) * MS] = res.results[core]["out_s"]
    return out.reshape(B, C, H, W)


# revision 9
# speedup vs baseline: 1.0707x; 1.0190x over previous
"""Trainium2 Bass kernel for nn_CrossAttention_9174050144362.

Reference computation (per batch b, spatial flattened to hw=4096):
    Q = Wq @ a + bq      [128, 4096]
    K = Wk @ p + bk      [128, 4096]
    V = Wv @ p + bv      [256, 4096]
    attn = softmax_n(Q^T K)            [4096, 4096]
    out  = V @ attn^T + a              [256, 4096]

Sharding: 8 cores = (4 batches) x (2 query halves of 2048). Each core
computes full K/V for its batch (recomputed per half; ~6% extra flops)
and attends its 2048 queries against all 4096 keys. No collectives.

Per-core kernel strategy (compute-bound, PE-centric):
  * All matmuls in float32r (FP22 truncated fp32) -> 1 cycle/row on PE.
  * S is computed TRANSPOSED: S^T[n, m] tiles via matmul(lhsT=K_tile,
    rhs=Q_chunk), so the softmax reduction over n is a partition-dim
    reduction. P^T = exp(S^T) goes straight from PSUM through ACT to
    SBUF, and feeds matmul(lhsT=V^T_tile, rhs=P^T) accumulation - no
    transposes of the big 4096x2048 matrix anywhere.
  * The softmax denominator is obtained by accumulating P^T tiles on
    the (otherwise idle) Vector engine, then one matmul with an
    all-ones [128,128] lhsT which simultaneously reduces over the
    partition dim and broadcasts the row-sums to all 128 partitions.
  * bv is folded into the epilogue: sum_n attn = 1, so V-bias adds
    bv[c] to every output pixel (a must stay pristine for the Q proj).
"""

import numpy as np

import concourse.bass as bass
import concourse.tile as tile
from concourse import bacc, mybir
from concourse.bass_utils import run_bass_kernel_spmd

B, C, H, W = 4, 256, 64, 64
HW = H * W            # 4096 keys
CH = C // 2           # 128 q/k channels
P = 128               # partitions
MS = HW // 2          # 2048 queries per core
MCH = 512             # query chunk (PSUM-bank sized)
NT = HW // P          # 32 key tiles
NCORES = 8

F32 = mybir.dt.float32
F32R = mybir.dt.float32r
BF16 = mybir.dt.bfloat16
AF = mybir.ActivationFunctionType

# Module-level knobs for the dev harness (test.py); harmless defaults for
# the grading path which just calls kernel(**inputs).
TRACE = False
TMPDIR = None
LAST_RESULT = None

_PROG = None
_ONES = np.ones((P, P), dtype=np.float32)


def _emit(tc, out_d, a_d, p_d, wqt_d, wkt_d, wvt_d, bq_d, bk_d, bv_d, ones_d):
    nc = tc.nc
    ts = bass.ts

    with (
        tc.tile_pool(name="statics", bufs=1) as statics,
        tc.tile_pool(name="ptp", bufs=6) as ptp,
        tc.tile_pool(name="accp", bufs=2) as accp,
        tc.tile_pool(name="rcp", bufs=2) as rcp,
        tc.tile_pool(name="osb", bufs=3) as osb,
        tc.tile_pool(name="psA", bufs=3, space="PSUM") as psA,
        tc.tile_pool(name="psOut", bufs=2, space="PSUM") as psOut,
        tc.tile_pool(name="psDen", bufs=1, space="PSUM") as psDen,
    ):
        # ---- statics into SBUF (c split as c = co*128 + ci, ci on partitions)
        # a/p are DMA'd in m-chunks so projections can start before the full
        # tensor lands (Tile tracks subtile deps).
        # small weight/bias DMAs FIRST so projections unblock within ~1us;
        # the bulk a/p loads stream behind them in m-chunks (subtile deps).
        wqt_sb = statics.tile([P, 2, CH], F32R)
        nc.sync.dma_start(wqt_sb[:], wqt_d.rearrange("(co ci) o -> ci co o", ci=P))
        wkt_sb = statics.tile([P, 2, CH], F32R)
        nc.sync.dma_start(wkt_sb[:], wkt_d.rearrange("(co ci) o -> ci co o", ci=P))
        wvt_sb = statics.tile([P, 2, C], F32R)
        nc.sync.dma_start(wvt_sb[:], wvt_d.rearrange("(co ci) o -> ci co o", ci=P))
        bq_sb = statics.tile([P, 1], F32)
        nc.sync.dma_start(bq_sb[:], bq_d[:])
        bk_sb = statics.tile([P, 1], F32)
        nc.sync.dma_start(bk_sb[:], bk_d[:])
        bv_sb = statics.tile([P, 2], F32)
        nc.sync.dma_start(bv_sb[:], bv_d[:])
        ones_sb = statics.tile([P, P], F32R)
        nc.sync.dma_start(ones_sb[:], ones_d[:])
        a_v = a_d.rearrange("(co ci) m -> ci co m", ci=P)
        a_sb = statics.tile([P, 2, MS], F32R)
        for h in range(4):
            nc.scalar.dma_start(a_sb[:, :, ts(h, MS // 4)], a_v[:, :, ts(h, MS // 4)])
        p_v = p_d.rearrange("(co ci) m -> ci co m", ci=P)
        p_sb = statics.tile([P, 2, HW], F32R)
        for h in range(8):
            nc.gpsimd.dma_start(p_sb[:, :, ts(h, HW // 8)], p_v[:, :, ts(h, HW // 8)])

        # ---- projections
        q_sb = statics.tile([P, MS], F32R)
        k_sb = statics.tile([P, HW], F32R)
        vt_sb = statics.tile([P, NT, C], BF16)

        for t in range(MS // MCH):
            ps_q = psA.tile([P, MCH], F32, tag="ps")
            for co in range(2):
                nc.tensor.matmul(
                    ps_q[:], wqt_sb[:, co, :], a_sb[:, co, ts(t, MCH)],
                    start=(co == 0), stop=(co == 1),
                )
            nc.scalar.activation(
                q_sb[:, ts(t, MCH)], ps_q[:], AF.Identity, bias=bq_sb[:, 0:1]
            )
        for t in range(HW // MCH):
            ps_k = psA.tile([P, MCH], F32, tag="ps")
            for co in range(2):
                nc.tensor.matmul(
                    ps_k[:], wkt_sb[:, co, :], p_sb[:, co, ts(t, MCH)],
                    start=(co == 0), stop=(co == 1),
                )
            nc.scalar.activation(
                k_sb[:, ts(t, MCH)], ps_k[:], AF.Identity, bias=bk_sb[:, 0:1]
            )
        # V^T tiles: vt[n, c] = sum_ci p[ci, n] * WvT[ci, c]   (no bias)
        for t in range(NT):
            ps_v = psA.tile([P, C], F32, tag="ps")
            for co in range(2):
                nc.tensor.matmul(
                    ps_v[:], p_sb[:, co, ts(t, P)], wvt_sb[:, co, :],
                    start=(co == 0), stop=(co == 1),
                )
            nc.scalar.copy(vt_sb[:, t, :], ps_v[:])

        out_v = out_d.rearrange("(co ci) m -> ci co m", ci=P)

        def epilogue_den(acc):
            # denominator: ones^T @ acc reduces over n AND broadcasts to
            # all partitions in a single matmul
            den_ps = psDen.tile([P, MCH], F32, tag="den")
            nc.tensor.matmul(den_ps[:], ones_sb[:], acc[:], start=True, stop=True)
            recip = rcp.tile([P, MCH], F32, tag="rc")
            nc.vector.reciprocal(recip[:], den_ps[:])
            return recip

        def epilogue_out(mc, out_ps, recip):
            for co in range(2):
                o_sb = osb.tile([P, MCH], F32, tag="osb")
                nc.vector.tensor_mul(o_sb[:], out_ps[:, co, :], recip[:])
                nc.vector.tensor_add(o_sb[:], o_sb[:], a_sb[:, co, ts(mc, MCH)])
                # bv folded here: attn rows sum to 1, so V-bias is +bv[c]
                nc.vector.tensor_scalar_add(o_sb[:], o_sb[:], bv_sb[:, co : co + 1])
                nc.sync.dma_start(out_v[:, co, ts(mc, MCH)], o_sb[:])

        # ---- attention main loop over query chunks. Chunk k's
        # denominator matmul+reciprocal run early in chunk k+1 (t==2), its
        # normalize/store runs at t==6, so the PE stream never stalls on
        # the DVE accumulation chain; the final chunk pays a short tail.
        pending_den = None
        pending_out = None
        for mc in range(MS // MCH):
            out_ps = psOut.tile([P, 2, MCH], F32, tag="out")
            acc = accp.tile([P, MCH], F32R, tag="acc")
            prev_pt = None
            for t in range(NT):
                s_ps = psA.tile([P, MCH], F32, tag="ps")
                nc.tensor.matmul(
                    s_ps[:], k_sb[:, ts(t, P)], q_sb[:, ts(mc, MCH)],
                    start=True, stop=True,
                )
                pt = ptp.tile([P, MCH], BF16, tag="pt")
                nc.scalar.activation(pt[:], s_ps[:], AF.Exp)
                if t == 0:
                    nc.vector.tensor_copy(acc[:], pt[:])
                else:
                    nc.vector.tensor_add(acc[:], acc[:], pt[:])
                # V.P matmuls run one iteration behind the S matmul so the
                # PE never waits on the exp of the tile it just produced.
                if prev_pt is not None:
                    tp, pt_prev = prev_pt
                    for co in range(2):
                        nc.tensor.matmul(
                            out_ps[:, co, :], vt_sb[:, tp, ts(co, P)], pt_prev[:],
                            start=(tp == 0), stop=False,
                        )
                prev_pt = (t, pt)
                if t == 2 and pending_den is not None:
                    pmc, pout, pacc = pending_den
                    pending_out = (pmc, pout, epilogue_den(pacc))
                    pending_den = None
                if t == 6 and pending_out is not None:
                    epilogue_out(*pending_out)
                    pending_out = None
            tp, pt_prev = prev_pt
            for co in range(2):
                nc.tensor.matmul(
                    out_ps[:, co, :], vt_sb[:, tp, ts(co, P)], pt_prev[:],
                    start=False, stop=True,
                )
            pending_den = (mc, out_ps, acc)
        pmc, pout, pacc = pending_den
        epilogue_out(pmc, pout, epilogue_den(pacc))


def _build():
    nc = bacc.Bacc("TRN2", target_bir_lowering=False, debug=False)
    a_d = nc.dram_tensor("a_s", [C, MS], F32R, kind="ExternalInput").ap()
    p_d = nc.dram_tensor("p_s", [C, HW], F32R, kind="ExternalInput").ap()
    wqt_d = nc.dram_tensor("wqt", [C, CH], F32R, kind="ExternalInput").ap()
    wkt_d = nc.dram_tensor("wkt", [C, CH], F32R, kind="ExternalInput").ap()
    wvt_d = nc.dram_tensor("wvt", [C, C], F32R, kind="ExternalInput").ap()
    bq_d = nc.dram_tensor("bq", [CH, 1], F32, kind="ExternalInput").ap()
    bk_d = nc.dram_tensor("bk", [CH, 1], F32, kind="ExternalInput").ap()
    bv_d = nc.dram_tensor("bv", [P, 2], F32, kind="ExternalInput").ap()
    ones_d = nc.dram_tensor("onesm", [P, P], F32R, kind="ExternalInput").ap()
    out_d = nc.dram_tensor("out_s", [C, MS], F32, kind="ExternalOutput").ap()
    with tile.TileContext(nc) as tc:
        _emit(tc, out_d, a_d, p_d, wqt_d, wkt_d, wvt_d, bq_d, bk_d, bv_d, ones_d)
    nc.compile()
    return nc


def _get_prog():
    global _PROG
    if _PROG is None:
        _PROG = _build()
    return _PROG


def kernel(**inputs):
    a = np.ascontiguousarray(np.asarray(inputs["a"], dtype=np.float32)).reshape(
        B, C, HW
    )
    p = np.ascontiguousarray(np.asarray(inputs["p"], dtype=np.float32)).reshape(
        B, C, HW
    )
    wqt = np.ascontiguousarray(np.asarray(inputs["Wq"], dtype=np.float32).T)
    wkt = np.ascontiguousarray(np.asarray(inputs["Wk"], dtype=np.float32).T)
    wvt = np.ascontiguousarray(np.asarray(inputs["Wv"], dtype=np.float32).T)
    bq = np.ascontiguousarray(np.asarray(inputs["bq"], dtype=np.float32)).reshape(
        CH, 1
    )
    bk = np.ascontiguousarray(np.asarray(inputs["bk"], dtype=np.float32)).reshape(
        CH, 1
    )
    bv = np.ascontiguousarray(
        np.asarray(inputs["bv"], dtype=np.float32).reshape(2, P).T
    )

    nc = _get_prog()
    in_maps = []
    for core in range(NCORES):
        b, h = divmod(core, 2)
        in_maps.append(
            {
                "a_s": np.ascontiguousarray(a[b, :, h * MS : (h + 1) * MS]),
                "p_s": p[b],
                "wqt": wqt,
                "wkt": wkt,
                "wvt": wvt,
                "bq": bq,
                "bk": bk,
                "bv": bv,
                "onesm": _ONES,
            }
        )
    kwargs = {}
    if TRACE:
        kwargs["trace"] = True
        if TMPDIR:
            kwargs["tmpdir"] = TMPDIR
    res = run_bass_kernel_spmd(nc, in_maps, core_ids=list(range(NCORES)), **kwargs)
    global LAST_RESULT
    LAST_RESULT = res

    out = np.empty((B, C, HW), dtype=np.float32)
    for core in range(NCORES):
        b, h = divmod(core, 2)
        out[b, :, h * MS : (h + 1) * MS] = res.results[core]["out_s"]
    return out.reshape(B, C, H, W)
